# revision 10
# baseline (speedup 1.0000x reference)
"""Trainium2 Bass kernel for nn_PostProcessCocoGrounding.

Pipeline per core (32 images):
  host: shard batch over 8 cores, pre-transpose pred_logits to [img, T, Q]
  device, per image:
    sigmoid (ACT) -> prob^T = pmT.T @ sigT (PE, fp32, K=256 in 2 halves)
    -> evac PSUM->SBUF (ACT) -> PE transpose 128-query chunks -> [128, 728]
       PSUM layout (partition = q%128, free = (q//128)*91 + c)
    -> per-partition top-8 of each half (DVE max/max_index, read PSUM)
  merge, all images batched:
    DMA relayout candidates -> [4*img+quarter, 512], 14 rounds of
    max/max_index/match_replace -> 4 sorted streams of 112 per image
    -> DMA -> [img, 512] -> values-only bitonic merge-4 -> sorted top-300
    -> max_index search to recover positions -> gpsimd indirect_copy chain
       to decode (q, c) -> gather pre-converted scaled boxes
"""
import numpy as np
from concurrent.futures import ThreadPoolExecutor

B, Q, T, C = 256, 900, 256, 91
NCORES = 8
NIMG = B // NCORES      # 32 images per core
NSEL = 300
HALF = 364              # scan half width = 4 chunks * 91
RND = 14                # extraction rounds (covers quarter membership <= 101)
STRLEN = 128            # padded extracted-stream length (112 real + pad)
SRCH = 304              # search width = 38 groups * 8

_NC_CACHE = {}
# stream position i holding output rank m: m = 19*(i%16) + i//16  =>  i(m) = 16*(m%19) + m//19
_M = np.arange(SRCH)
STREAM_PERM = (16 * (_M % 19) + _M // 19).astype(np.int64)


def build_kernel(nimg=NIMG, debug=False):
    import concourse.bacc as bacc
    import concourse.mybir as mybir
    from concourse.tile import TileContext
    from concourse.masks import make_identity

    f32 = mybir.dt.float32
    u16 = mybir.dt.uint16
    i32 = mybir.dt.int32
    AF = mybir.ActivationFunctionType
    OP = mybir.AluOpType

    ngrp = (nimg + 7) // 8          # indirect_copy batches of 8 images
    nc = bacc.Bacc("TRN2", target_bir_lowering=False, debug=False)

    lgt = nc.declare_dram_parameter("lgt", [nimg, T, Q], f32, isOutput=False)
    pmt = nc.declare_dram_parameter("pmt", [T, C], f32, isOutput=False)
    bxi = nc.declare_dram_parameter("bx", [nimg, Q, 4], f32, isOutput=False)
    tsi = nc.declare_dram_parameter("ts", [nimg, 2], f32, isOutput=False)
    sco_o = nc.declare_dram_parameter("scores", [nimg, NSEL], f32, isOutput=True)
    lab_o = nc.declare_dram_parameter("labels", [nimg, NSEL], i32, isOutput=True)
    box_o = nc.declare_dram_parameter("boxes", [nimg, SRCH, 4], f32, isOutput=True)
    dbg = {}
    if debug:
        def dbg_out(name, shape, dt=f32):
            dbg[name] = nc.declare_dram_parameter(name, shape, dt, isOutput=True)
        dbg_out("d_mallv", [128, 16 * nimg])
        dbg_out("d_malli", [128, 16 * nimg], u16)
        dbg_out("d_exv", [4 * nimg, 512])
        dbg_out("d_ev", [4 * nimg, STRLEN])
        dbg_out("d_ep", [4 * nimg, STRLEN], u16)
        dbg_out("d_fv", [nimg, 512])
        dbg_out("d_srt", [nimg, 512])
        dbg_out("d_fvpos", [nimg, SRCH], u16)
        dbg_out("d_expos", [nimg, SRCH], u16)
        dbg_out("d_tkcol", [nimg, SRCH], u16)
        dbg_out("d_qq", [nimg, SRCH], u16)
        dbg_out("d_obx", [nimg, Q, 4])

    with TileContext(nc) as tc:
        with (
            tc.tile_pool(name="cst", bufs=1) as cst,
            tc.tile_pool(name="pin", bufs=3) as pin,
            tc.tile_pool(name="pw", bufs=2) as pw,
            tc.tile_pool(name="pmg", bufs=1) as pmg,
            tc.tile_pool(name="psA", bufs=2, space="PSUM") as psA,
            tc.tile_pool(name="psB", bufs=2, space="PSUM") as psB,
        ):
            ident = cst.tile([128, 128], f32, tag="ident")
            make_identity(nc, ident)
            pmT = cst.tile([128, 2, C], f32, tag="pmT")
            nc.sync.dma_start(out=pmT, in_=pmt.rearrange("(h p) c -> p h c", p=128))

            # persistent candidate tiles
            mallv = pmg.tile([128, 16 * nimg], f32, tag="mallv")
            malli = pmg.tile([128, 16 * nimg], u16, tag="malli")

            # ---------------- per-image loop ----------------
            for i in range(nimg):
                lg = pin.tile([128, 2, Q], f32, tag="lg")
                nc.sync.dma_start(out=lg, in_=lgt[i].rearrange("(h p) q -> p h q", p=128))
                # sigmoid in place
                nc.scalar.activation(lg, lg, AF.Sigmoid)

                # prob^T = pmT.T @ sig : [91, 900] in two PSUM banks
                pa1 = psA.tile([C, 512], f32, tag="pa1")
                pa2 = psA.tile([C, Q - 512], f32, tag="pa2")
                for h in range(2):
                    nc.tensor.matmul(pa1, pmT[:, h, :], lg[:, h, 0:512],
                                     start=(h == 0), stop=(h == 1))
                for h in range(2):
                    nc.tensor.matmul(pa2, pmT[:, h, :], lg[:, h, 512:Q],
                                     start=(h == 0), stop=(h == 1))

                # evac to SBUF [91, 1024]; cols 900:1024 = -1 padding
                probT = pw.tile([C, 1024], f32, tag="probT")
                nc.gpsimd.memset(probT[:, Q:1024], -1.0)
                nc.scalar.activation(probT[:, 0:512], pa1, AF.Copy)
                nc.scalar.activation(probT[:, 512:Q], pa2, AF.Copy)

                # PE transpose chunks of 128 queries -> [128, 2x364] PSUM
                pb1 = psB.tile([128, HALF], f32, tag="pb1")
                pb2 = psB.tile([128, HALF], f32, tag="pb2")
                for k in range(8):
                    dst = pb1 if k < 4 else pb2
                    off = (k % 4) * C
                    nc.tensor.transpose(dst[:, off:off + C],
                                        probT[:, 128 * k:128 * (k + 1)],
                                        ident[0:C, 0:C])

                # scan: per-partition top-8 of each half (reads PSUM)
                nc.vector.max(out=mallv[:, 16 * i:16 * i + 8], in_=pb1)
                nc.vector.max_index(out=malli[:, 16 * i:16 * i + 8],
                                    in_max=mallv[:, 16 * i:16 * i + 8], in_values=pb1)
                nc.vector.max(out=mallv[:, 16 * i + 8:16 * i + 16], in_=pb2)
                nc.vector.max_index(out=malli[:, 16 * i + 8:16 * i + 16],
                                    in_max=mallv[:, 16 * i + 8:16 * i + 16], in_values=pb2)

            # ---------------- merge stage ----------------
            # half-2 indices += HALF so TK col in [0, 728)
            half2 = malli.rearrange("p (i x) -> p i x", x=16)[:, :, 8:16]
            nc.vector.tensor_scalar(half2, half2, HALF, scalar2=None, op0=OP.add)
            if debug:
                nc.sync.dma_start(out=dbg["d_mallv"][:, :], in_=mallv)
                nc.sync.dma_start(out=dbg["d_malli"][:, :], in_=malli)

            # relayout via DRAM bounce: EXV[4i+Qr, 16s+j] = mallv[32Qr+s, 16i+j]
            bv = nc.dram_tensor("bv", [4 * nimg, 512], f32)
            bi = nc.dram_tensor("bi", [nimg, 2048], u16)
            for Qr in range(4):
                src_v = mallv[32 * Qr:32 * (Qr + 1), :].rearrange("s (i j) -> s i j", j=16)
                dst_v = bv.rearrange("(i q) (s j) -> q s i j", q=4, j=16)[Qr]
                nc.sync.dma_start(out=dst_v, in_=src_v[:, 0:nimg])
                src_i = malli[32 * Qr:32 * (Qr + 1), :].rearrange("s (i j) -> s i j", j=16)
                dst_i = bi.rearrange("i (q s j) -> q s i j", q=4, j=16)[Qr]
                nc.sync.dma_start(out=dst_i, in_=src_i[:, 0:nimg])
            exv = pmg.tile([128, 512], f32, tag="exv")
            fexi = pmg.tile([nimg, 2048], u16, tag="fexi")
            nc.sync.dma_start(out=exv[0:4 * nimg, :], in_=bv[:, :])
            nc.sync.dma_start(out=fexi, in_=bi[:, :])
            if debug:
                nc.sync.dma_start(out=dbg["d_exv"][:, :], in_=exv[0:4 * nimg, :])

            # extraction: RND rounds of top-8 per (image, quarter)
            ev = pmg.tile([128, STRLEN], f32, tag="ev")
            ep = pmg.tile([128, STRLEN], u16, tag="ep")
            nc.gpsimd.memset(ev[:, 8 * RND:STRLEN], -3.0)
            nc.gpsimd.memset(ep[:, 8 * RND:STRLEN], 0)
            exv_act = exv[0:4 * nimg, :]
            for r in range(RND):
                nc.vector.max(out=ev[0:4 * nimg, 8 * r:8 * r + 8], in_=exv_act)
                nc.vector.max_index(out=ep[0:4 * nimg, 8 * r:8 * r + 8],
                                    in_max=ev[0:4 * nimg, 8 * r:8 * r + 8], in_values=exv_act)
                if r + 1 < RND:
                    nc.vector.match_replace(out=exv_act,
                                            in_to_replace=ev[0:4 * nimg, 8 * r:8 * r + 8],
                                            in_values=exv_act, imm_value=-2.0)
            if debug:
                nc.sync.dma_start(out=dbg["d_ev"][:, :], in_=ev[0:4 * nimg, :])
                nc.sync.dma_start(out=dbg["d_ep"][:, :], in_=ep[0:4 * nimg, :])

            # relayout streams: FV[i, 128*Qr + r] = ev[4i+Qr, r]
            # (4i+Qr, r) partition-major flat order == fv flat order -> plain bounce
            bf = nc.dram_tensor("bf", [nimg, 512], f32)
            bp = nc.dram_tensor("bp", [nimg, 512], u16)
            nc.sync.dma_start(out=bf.rearrange("i (q r) -> (i q) r", q=4), in_=ev[0:4 * nimg, :])
            nc.sync.dma_start(out=bp.rearrange("i (q r) -> (i q) r", q=4), in_=ep[0:4 * nimg, :])
            fv = pmg.tile([nimg, 512], f32, tag="fv")
            fp = pmg.tile([nimg, 512], u16, tag="fp")
            nc.sync.dma_start(out=fv, in_=bf[:, :])
            nc.sync.dma_start(out=fp, in_=bp[:, :])
            if debug:
                nc.sync.dma_start(out=dbg["d_fv"][:, :], in_=fv)

            # values-only bitonic merge (descending), ping-pong b1 <-> b2
            b1 = pmg.tile([nimg, 512], f32, tag="b1")
            b2 = pmg.tile([nimg, 512], f32, tag="b2")

            def cross_stage(src, dst, L, nblk):
                # dst[b*2L + k] = max(src[b*2L+k], src[b*2L + 2L-1-k]);
                # dst[b*2L + L + k] = min of same pair (s-space bitonic stage 1)
                s_lo = src.rearrange("p (b x) -> p b x", x=2 * L)[:, :, 0:L]
                s_hi_rev = src.rearrange("p (b x) -> p b x", x=2 * L)[:, :, 2 * L - 1:L - 1:-1]
                d_lo = dst.rearrange("p (b x) -> p b x", x=2 * L)[:, :, 0:L]
                d_hi = dst.rearrange("p (b x) -> p b x", x=2 * L)[:, :, L:2 * L]
                nc.vector.tensor_tensor(d_lo, s_lo, s_hi_rev, OP.max)
                nc.vector.tensor_tensor(d_hi, s_lo, s_hi_rev, OP.min)

            def dist_stage(src, dst, d):
                v = src.rearrange("p (b two m) -> p b two m", two=2, m=d)
                o = dst.rearrange("p (b two m) -> p b two m", two=2, m=d)
                nc.vector.tensor_tensor(o[:, :, 0, :], v[:, :, 0, :], v[:, :, 1, :], OP.max)
                nc.vector.tensor_tensor(o[:, :, 1, :], v[:, :, 0, :], v[:, :, 1, :], OP.min)

            cur, nxt = fv, b1
            cross_stage(cur, nxt, 128, 2); cur, nxt = b1, b2
            for d in (64, 32, 16, 8, 4, 2, 1):
                dist_stage(cur, nxt, d); cur, nxt = nxt, cur
            cross_stage(cur, nxt, 256, 1); cur, nxt = nxt, cur
            for d in (128, 64, 32, 16, 8, 4, 2, 1):
                dist_stage(cur, nxt, d); cur, nxt = nxt, cur
            srt = cur     # fully sorted descending [nimg, 512]
            if debug:
                nc.sync.dma_start(out=dbg["d_srt"][:, :], in_=srt)
            nc.sync.dma_start(out=sco_o[:, :], in_=srt[:, 0:NSEL])

            # search: position of each sorted value in FV
            fvpos = pmg.tile([nimg, SRCH], u16, tag="fvpos")
            for g in range(SRCH // 8):
                nc.vector.max_index(out=fvpos[:, 8 * g:8 * g + 8],
                                    in_max=srt[:, 8 * g:8 * g + 8], in_values=fv)
            if debug:
                nc.sync.dma_start(out=dbg["d_fvpos"][:, :], in_=fvpos)

            # ---- gather chain (indirect_copy: 8 images per call) ----
            # indirect_copy consumes its per-group index list wrapped 16-way:
            # stream position i uses idxs[16g + i%16, i//16].  We load the
            # wrapped tile as a STRAIGHT reshape of a [SRCH] row (iw[16g+p, s]
            # = row[19p+s]), so stream i gathers row[m(i)], m(i)=19*(i%16)+i//16.
            # All downstream arrays therefore hold content permuted by m();
            # elementwise decode is permutation-invariant and we un-permute
            # with strided engine APs right before output.
            SS = SRCH // 16     # 19

            _wbounce = {}

            def bounce_rows(row_src, name):
                bw = nc.dram_tensor(name, [nimg, SRCH], u16)
                nc.sync.dma_start(out=bw[:, :], in_=row_src)
                _wbounce[name] = bw

            def load_wrapped(name, k, tag):
                # iw[16g+p, s] = rows[8k+g, 19p+s]: flat reshape of 8 rows
                iw = pw.tile([128, SS], u16, tag=tag)
                src = _wbounce[name][8 * k:8 * k + 8].rearrange("g (p s) -> (g p) s", s=SS)
                nc.sync.dma_start(out=iw, in_=src)
                return iw

            # EXPOS_perm = FP[i][FVPOS[m(i)]]
            expos = pmg.tile([nimg, SRCH], u16, tag="expos")
            bounce_rows(fvpos, "bw1")
            fpd = pmg.tile([128, 512], u16, tag="fpd")
            nc.vector.memset(fpd, 0)
            for k in range(ngrp):
                iw = load_wrapped("bw1", k, "iw1")
                nc.sync.dma_start(out=fpd.rearrange("(g p) f -> g p f", p=16)[0:8, 0, :],
                                  in_=fp[8 * k:8 * k + 8, :])
                god = pw.tile([128, SRCH], u16, tag="god1")
                nc.gpsimd.indirect_copy(god, fpd, iw, i_know_ap_gather_is_preferred=True)
                nc.sync.dma_start(out=expos[8 * k:8 * k + 8, :],
                                  in_=god.rearrange("(g p) f -> g p f", p=16)[0:8, 0, :])
            if debug:
                nc.sync.dma_start(out=dbg["d_expos"][:, :], in_=expos)

            # fvposP[g, i] = fvpos[g, m(i)]
            fvposP = pmg.tile([nimg, SRCH], u16, tag="fvposP")
            nc.vector.tensor_copy(fvposP.rearrange("g (s p) -> g s p", p=16),
                                  fvpos.rearrange("g (p s) -> g s p", s=SS))

            # TKCOL_perm = FEXI[i][(FVPOSP>>7)*512 + EXPOS_perm]; index list must
            # be pre-wrapped: gidxW[g, 19p+s] = gidx_perm[g, 16s+p]
            gidx = pmg.tile([nimg, SRCH], u16, tag="gidx")
            nc.vector.tensor_scalar(gidx, fvposP, 7, scalar2=9,
                                    op0=OP.logical_shift_right, op1=OP.logical_shift_left)
            gidxW = pmg.tile([nimg, SRCH], u16, tag="gidxW")
            nc.vector.tensor_tensor(gidxW.rearrange("g (p s) -> g p s", s=SS),
                                    gidx.rearrange("g (s p) -> g p s", p=16),
                                    expos.rearrange("g (s p) -> g p s", p=16), OP.add)
            tkcol = pmg.tile([nimg, SRCH], u16, tag="tkcol")
            bounce_rows(gidxW, "bw2")
            fxd = pmg.tile([128, 2048], u16, tag="fxd")
            nc.vector.memset(fxd, 0)
            for k in range(ngrp):
                iw = load_wrapped("bw2", k, "iw2")
                nc.sync.dma_start(out=fxd.rearrange("(g p) f -> g p f", p=16)[0:8, 0, :],
                                  in_=fexi[8 * k:8 * k + 8, :])
                god = pw.tile([128, SRCH], u16, tag="god2")
                nc.gpsimd.indirect_copy(god, fxd, iw, i_know_ap_gather_is_preferred=True)
                nc.sync.dma_start(out=tkcol[8 * k:8 * k + 8, :],
                                  in_=god.rearrange("(g p) f -> g p f", p=16)[0:8, 0, :])
            if debug:
                nc.sync.dma_start(out=dbg["d_tkcol"][:, :], in_=tkcol)

            # decode (all permuted content): p = ((FVPOSP>>7)<<5) + (EXPOS>>4);
            # chunk = TKCOL//91; c = TKCOL - 91*chunk; q = chunk*128 + p
            pp = pmg.tile([nimg, SRCH], u16, tag="pp")
            nc.vector.tensor_scalar(pp, fvposP, 7, scalar2=5,
                                    op0=OP.logical_shift_right, op1=OP.logical_shift_left)
            t0 = pmg.tile([nimg, SRCH], u16, tag="t0")
            nc.vector.tensor_scalar(t0, expos, 4, scalar2=None, op0=OP.logical_shift_right)
            nc.vector.tensor_tensor(pp, pp, t0, OP.add)
            colf = pmg.tile([nimg, SRCH], f32, tag="colf")
            nc.vector.tensor_copy(colf, tkcol)
            # (col - 45)/91 is within +-0.495 of col//91 => exact under HW round-to-nearest cast
            nc.vector.tensor_scalar(colf, colf, -45.0, scalar2=1.0 / C, op0=OP.add, op1=OP.mult)
            chunk = pmg.tile([nimg, SRCH], u16, tag="chunk")
            nc.vector.tensor_copy(chunk, colf)      # cast toward int
            cc = pmg.tile([nimg, SRCH], u16, tag="cc")
            nc.vector.tensor_scalar(cc, chunk, C, scalar2=None, op0=OP.mult)
            nc.vector.tensor_tensor(cc, tkcol, cc, OP.subtract)
            qq = pmg.tile([nimg, SRCH], u16, tag="qq")
            nc.vector.tensor_scalar(qq, chunk, 7, scalar2=None, op0=OP.logical_shift_left)
            nc.vector.tensor_tensor(qq, qq, pp, OP.add)
            if debug:
                nc.sync.dma_start(out=dbg["d_qq"][:, :], in_=qq)
            # un-permute labels: lab32[g, 19p+s] = cc[g, 16s+p]
            lab32 = pmg.tile([nimg, SRCH], i32, tag="lab32")
            nc.vector.tensor_copy(lab32.rearrange("g (p s) -> g p s", s=SS),
                                  cc.rearrange("g (s p) -> g p s", p=16))
            nc.sync.dma_start(out=lab_o[:, :], in_=lab32[:, 0:NSEL])

            # ---- boxes: convert + scale all, then gather by q ----
            ts_t = pmg.tile([nimg, 2], f32, tag="ts_t")
            nc.sync.dma_start(out=ts_t, in_=tsi[:, :])
            scl4 = pmg.tile([nimg, 4], f32, tag="scl4")
            for cdx, src in enumerate([1, 0, 1, 0]):   # [w, h, w, h]
                nc.vector.tensor_copy(scl4[:, cdx:cdx + 1], ts_t[:, src:src + 1])
            bt = pmg.tile([nimg, Q, 4], f32, tag="bt")
            nc.sync.dma_start(out=bt, in_=bxi[:, :, :])
            obx = pmg.tile([nimg, Q, 4], f32, tag="obx")
            hw_ = pmg.tile([nimg, Q, 2], f32, tag="hw_")
            nc.vector.tensor_scalar_mul(hw_, bt[:, :, 2:4], 0.5)
            nc.vector.tensor_tensor(obx[:, :, 0:2], bt[:, :, 0:2], hw_, OP.subtract)
            nc.vector.tensor_tensor(obx[:, :, 2:4], bt[:, :, 0:2], hw_, OP.add)
            for cdx in range(4):
                nc.vector.tensor_scalar(obx[:, :, cdx], obx[:, :, cdx],
                                        scl4[:, cdx:cdx + 1], scalar2=None, op0=OP.mult)
            if debug:
                nc.sync.dma_start(out=dbg["d_obx"][:, :, :], in_=obx)

            # gather 4-vectors: pre-wrapped idx = q*4
            q4w = pmg.tile([nimg, SRCH], u16, tag="q4w")
            nc.vector.tensor_scalar(q4w.rearrange("g (p s) -> g p s", s=SS),
                                    qq.rearrange("g (s p) -> g p s", p=16),
                                    2, scalar2=None, op0=OP.logical_shift_left)
            bounce_rows(q4w, "bw3")
            bxd = pmg.tile([128, Q, 4], f32, tag="bxd")
            nc.vector.memset(bxd, 0)
            for k in range(ngrp):
                iw = load_wrapped("bw3", k, "iw3")
                nc.sync.dma_start(out=bxd.rearrange("(g p) q c -> g p q c", p=16)[0:8, 0, :, :],
                                  in_=obx[8 * k:8 * k + 8])
                # component-wise 2D gathers (3D indirect_copy fails ISA check):
                # same idx list (4q), data offset by c selects component c
                gbo = pw.tile([128, SRCH, 4], f32, tag="gbo")
                datf = bxd.rearrange("p q c -> p (q c)")
                for cdx in range(4):
                    gbc = pw.tile([128, SRCH], f32, tag="gbc")
                    nc.gpsimd.indirect_copy(gbc, datf[:, cdx:Q * 4], iw,
                                            i_know_ap_gather_is_preferred=True)
                    nc.vector.tensor_copy(gbo[:, :, cdx], gbc)
                # boxes leave in stream (permuted) order; host un-permutes
                nc.sync.dma_start(
                    out=box_o[8 * k:8 * k + 8],
                    in_=gbo.rearrange("(g p) s c -> g p s c", p=16)[0:8, 0, :, :])
    nc.compile()
    return nc


def _get_nc():
    if "nc" not in _NC_CACHE:
        _NC_CACHE["nc"] = build_kernel(NIMG, debug=False)
    return _NC_CACHE["nc"]


def _marshal_core(args):
    pl_c, bx_c, ts_c, pmt = args
    lgt = np.ascontiguousarray(pl_c.transpose(0, 2, 1))   # [nimg, T, Q]
    return {"lgt": lgt, "pmt": pmt, "bx": np.ascontiguousarray(bx_c),
            "ts": np.ascontiguousarray(ts_c)}


def make_in_maps(pred_logits, pred_boxes, positive_map, target_sizes):
    pl = np.asarray(pred_logits, dtype=np.float32)
    bx = np.asarray(pred_boxes, dtype=np.float32)
    pm = np.asarray(positive_map, dtype=np.float32)
    ts = np.asarray(target_sizes, dtype=np.float32)
    pmt = np.ascontiguousarray(pm.T)                      # [T, C]
    jobs = [(pl[c * NIMG:(c + 1) * NIMG], bx[c * NIMG:(c + 1) * NIMG],
             ts[c * NIMG:(c + 1) * NIMG], pmt) for c in range(NCORES)]
    with ThreadPoolExecutor(NCORES) as ex:
        return list(ex.map(_marshal_core, jobs))


def kernel(pred_logits, pred_boxes, positive_map, target_sizes, num_select):
    assert int(num_select) == NSEL
    from concourse.bass_utils import run_bass_kernel_spmd
    in_maps = make_in_maps(pred_logits, pred_boxes, positive_map, target_sizes)
    nc = _get_nc()
    res = run_bass_kernel_spmd(nc, in_maps, core_ids=list(range(NCORES))).results
    scores = np.concatenate([r["scores"] for r in res], axis=0)
    labels = np.concatenate([r["labels"] for r in res], axis=0)
    boxes_raw = np.concatenate([r["boxes"] for r in res], axis=0)
    boxes = boxes_raw[:, STREAM_PERM[:NSEL], :]
    return scores, labels, boxes
